# revision 33
# baseline (speedup 1.0000x reference)
"""AnglePotentials on 8 Trainium2 NeuronCores.

Math: for each angle (i, i+1, i+2) the energy term depends only on the base
atom index i, so we precompute on-device a per-atom table
    u[j] = (arccos(cos_angle(j)) - thetao)^2
from xyz (streamed, elementwise). Since bond vectors are shared between
neighbouring triplets (D[j] = xyz[j]-xyz[j+1] feeds both bv1[j] and bv2[j-1]),
one difference/wrap/square stream serves both bond vectors.

The energy is the sum of u over the 4M base indices. Rather than a
per-element gather (slow on this hardware: GPSIMD ap_gather ~27ns/idx,
indirect DMA descriptors are per-dest-row), angles are bucketed on the host
into W-atom cells (cell-list sharding); the device evaluates, for each
window position w in [0,W), one fused scalar_tensor_tensor op
    (slot == w) * u[cell, w]   with fused accumulate-sum (accum_out)
over all angle slots.

Sharding: atoms are range-partitioned over the 8 cores (262144 per core,
2048 per SBUF partition); each core computes a partial energy and the host
sums the 8 partials. Work that is off the DVE critical path (the bond
difference D and the wrap-sign combine) runs on GPSIMD; transcendentals and
squares run on the Scalar engine.
"""

import numpy as np

# ---------------------------------------------------------------- geometry
N_ATOMS = 2_000_000
N_ANGLES = 4_000_000
BOXH = 25.0              # half box
A = 2048                 # atoms per partition
R = 128 * A              # atoms per core = 262144
N_CORES = 8
W = 4                    # cell width (atoms)
NBIN = A // W            # cells per partition = 512
L = 25                   # angle slots per cell (fixed-seed max is 25)
SENT = 100.0             # sentinel slot value (never matches a window pos)
NSTRIPE = 4              # phase-1 stripes
SA = A // NSTRIPE        # atoms per partition per stripe = 1024
XLEN = 3 * (R + 2)       # xyz floats per core shard

_nc_cache = [None]
DEBUG_DUMP = False


def _build():
    import concourse.bass as bass
    import concourse.bacc as bacc
    import concourse.mybir as mybir
    import concourse.tile as tile

    AF = mybir.ActivationFunctionType
    ALU = mybir.AluOpType
    f32 = mybir.dt.float32
    bf16 = mybir.dt.bfloat16

    nc = bacc.Bacc("TRN2", target_bir_lowering=False, debug=False,
                   num_devices=N_CORES)
    xyz_d = nc.dram_tensor("xyz", [XLEN], f32, kind="ExternalInput").ap()
    fl_d = nc.dram_tensor("fl", [128, NBIN * L], bf16, kind="ExternalInput").ap()
    k_d = nc.dram_tensor("k", [1, 1], f32, kind="ExternalInput").ap()
    th_d = nc.dram_tensor("thetao", [1, 1], f32, kind="ExternalInput").ap()
    out_d = nc.dram_tensor("out", [1, 1], f32, kind="ExternalOutput").ap()
    if DEBUG_DUMP:
        dbg_u = nc.dram_tensor("dbg_u", [128, A], f32, kind="ExternalOutput").ap()

    with tile.TileContext(nc) as tc:
        with (
            tc.tile_pool(name="persist", bufs=1) as persist,
            tc.tile_pool(name="work", bufs=2) as work,
            tc.tile_pool(name="big", bufs=1) as big,
            tc.tile_pool(name="small", bufs=1) as small,
            tc.tile_pool(name="psum", bufs=1, space="PSUM") as psum,
        ):
            u = persist.tile([128, A], f32)
            fl = persist.tile([128, NBIN * L], bf16)
            nc.sync.dma_start(fl[:], fl_d[:])

            # thetao -> per-partition bias = pi/2 - thetao
            th_s = persist.tile([1, 1], f32)
            nc.sync.dma_start(th_s[:], th_d[:])
            ones_row = persist.tile([1, 128], f32)
            nc.vector.memset(ones_row[:], 1.0)
            bias_ps = psum.tile([128, 1], f32)
            nc.tensor.matmul(bias_ps[:], ones_row[:], th_s[:], start=True, stop=True)
            bias_t = persist.tile([128, 1], f32)
            nc.vector.tensor_scalar(out=bias_t[:], in0=bias_ps[:],
                                    scalar1=-1.0, scalar2=float(np.pi / 2),
                                    op0=ALU.mult, op1=ALU.add)
            bias_m25 = persist.tile([128, 1], f32)
            nc.vector.memset(bias_m25[:], -BOXH)
            bias_p25 = persist.tile([128, 1], f32)
            nc.vector.memset(bias_p25[:], BOXH)

            cols = persist.tile([128, W], f32)

            # ---------------- phase 1: u[j] per stripe ----------------
            # D[j] = xyz[j] - xyz[j+1]; bv1[j] = wrap(D[j]); bv2[j] = -wrap(D[j+1])
            # dot' = sum_c Dw[j]*Dw[j+1] = -dot; nn[j] = |Dw[j]|^2
            # arccos = pi/2 + 2*arctan(dot'/(sqrt(q)+sqrt(m))), q = m - dot'^2
            for s in range(NSTRIPE):
                ND = 3 * SA + 3  # D elements (SA+1 atoms worth)
                X = work.tile([128, 3 * (SA + 2)], f32, tag="X")
                x_src = bass.AP(
                    tensor=xyz_d.tensor,
                    offset=s * 3 * SA,
                    ap=[[3 * A, 128], [1, 3 * (SA + 2)]],
                )
                nc.sync.dma_start(X[:], x_src)

                D = big.tile([128, ND], f32, tag="D")
                nc.gpsimd.tensor_tensor(out=D[:], in0=X[:, 0:ND],
                                        in1=X[:, 3:ND + 3], op=ALU.subtract)
                # wrap via sign: D - 25*(sign(D-25) + sign(D+25))
                s1 = big.tile([128, ND], f32, tag="s1")
                nc.scalar.activation(s1[:], D[:], AF.Sign, bias=bias_m25[:])
                s2 = big.tile([128, ND], f32, tag="s2")
                nc.scalar.activation(s2[:], D[:], AF.Sign, bias=bias_p25[:])
                nc.gpsimd.tensor_tensor(out=s1[:], in0=s1[:], in1=s2[:], op=ALU.add)
                Db = big.tile([128, ND], f32, tag="Db")
                nc.vector.scalar_tensor_tensor(out=Db[:], in0=s1[:], scalar=-BOXH,
                                               in1=D[:], op0=ALU.mult, op1=ALU.add)
                P = big.tile([128, 3 * SA], f32, tag="P")
                nc.vector.tensor_tensor(out=P[:], in0=Db[:, 0:3 * SA],
                                        in1=Db[:, 3:3 * SA + 3], op=ALU.mult)
                S = big.tile([128, ND], f32, tag="S")
                nc.scalar.activation(S[:], Db[:], AF.Square)

                def comp_sum(src_t, n, tag):
                    o = small.tile([128, n], f32, tag=tag)
                    v = src_t[:, 0:3 * n].rearrange("p (a c) -> p a c", c=3)
                    nc.vector.tensor_reduce(out=o[:], in_=v[:, :, :],
                                            axis=mybir.AxisListType.X, op=ALU.add)
                    return o

                dot = comp_sum(P, SA, "dot")      # dot' = -dot
                nn = comp_sum(S, SA + 1, "nn")    # |Dw|^2 per atom

                m = small.tile([128, SA], f32, tag="m")
                nc.vector.tensor_tensor(out=m[:], in0=nn[:, 0:SA], in1=nn[:, 1:SA + 1],
                                        op=ALU.mult)
                d2 = small.tile([128, SA], f32, tag="d2")
                nc.vector.tensor_tensor(out=d2[:], in0=dot[:], in1=dot[:], op=ALU.mult)
                q = small.tile([128, SA], f32, tag="q")
                nc.vector.tensor_tensor(out=q[:], in0=m[:], in1=d2[:], op=ALU.subtract)
                nc.vector.tensor_scalar(out=q[:], in0=q[:], scalar1=0.0,
                                        scalar2=None, op0=ALU.max)
                sq = small.tile([128, SA], f32, tag="sq")
                nc.scalar.activation(sq[:], q[:], AF.Sqrt)
                sm = small.tile([128, SA], f32, tag="sm")
                nc.scalar.activation(sm[:], m[:], AF.Sqrt)
                den = small.tile([128, SA], f32, tag="den")
                nc.vector.tensor_tensor(out=den[:], in0=sq[:], in1=sm[:], op=ALU.add)
                nc.vector.tensor_scalar(out=den[:], in0=den[:], scalar1=1e-30,
                                        scalar2=None, op0=ALU.max)
                rec = small.tile([128, SA], f32, tag="rec")
                nc.vector.reciprocal_approx_fast(rec[:], den[:])
                rho = small.tile([128, SA], f32, tag="rho")
                nc.vector.tensor_tensor(out=rho[:], in0=dot[:], in1=rec[:], op=ALU.mult)
                at = small.tile([128, SA], f32, tag="at")
                nc.scalar.activation(at[:], rho[:], AF.Arctan)
                # u = (arccos - thetao)^2 = (2*at' + (pi/2 - thetao))^2
                nc.scalar.activation(u[:, s * SA:(s + 1) * SA], at[:], AF.Square,
                                     bias=bias_t[:], scale=2.0)

            # ---------------- phase 2: fused compare-select-accumulate ----
            dummy = persist.tile([128, NBIN * L], bf16)
            fl2 = fl[:].rearrange("p (b t) -> p b t", t=L)
            u2 = u[:].rearrange("p (b w) -> p b w", w=W)
            d2v = dummy[:].rearrange("p (b t) -> p b t", t=L)
            for w in range(W):
                nc.vector.scalar_tensor_tensor(
                    out=d2v[:, :, :],
                    in0=fl2[:, :, :],
                    scalar=float(w),
                    in1=u2[:, :, w:w + 1].to_broadcast([128, NBIN, L]),
                    op0=ALU.is_equal,
                    op1=ALU.mult,
                    accum_out=cols[:, w:w + 1],
                )

            if DEBUG_DUMP:
                uf = persist.tile([128, A], f32)
                nc.vector.tensor_copy(uf[:], u[:])
                nc.sync.dma_start(dbg_u[:], uf[:])

            # ---------------- final reduction ----------------
            red = persist.tile([128, 1], f32)
            nc.vector.tensor_reduce(out=red[:], in_=cols[:],
                                    axis=mybir.AxisListType.X, op=ALU.add)
            ones_col = persist.tile([128, 1], f32)
            nc.vector.memset(ones_col[:], 1.0)
            tot_ps = psum.tile([1, 1], f32)
            nc.tensor.matmul(tot_ps[:], red[:], ones_col[:], start=True, stop=True)
            k_s = persist.tile([1, 1], f32)
            nc.sync.dma_start(k_s[:], k_d[:])
            tot = persist.tile([1, 1], f32)
            nc.vector.tensor_tensor(out=tot[:], in0=tot_ps[:], in1=k_s[:],
                                    op=ALU.mult)
            nc.vector.tensor_scalar(out=tot[:], in0=tot[:], scalar1=0.5,
                                    scalar2=None, op0=ALU.mult)
            nc.sync.dma_start(out_d[:], tot[:])
    nc.compile()
    return nc


def _shard_inputs(xyz, base, k, thetao):
    xyzf = np.ascontiguousarray(xyz, dtype=np.float32).reshape(-1)
    sb = np.sort(base.astype(np.int64), kind="stable")
    n = len(sb)
    nbins_g = N_CORES * 128 * NBIN
    edges = np.searchsorted(sb, np.arange(nbins_g + 1, dtype=np.int64) * W)
    g = sb // W
    r = np.arange(n, dtype=np.int64) - edges[g]
    if r.max(initial=0) >= L:
        return None  # cell overflow -> caller falls back
    import ml_dtypes
    bf = ml_dtypes.bfloat16
    slots = np.full((nbins_g, L), SENT, bf)
    slots[g, r] = (sb % W).astype(bf)
    slots = slots.reshape(N_CORES, 128, NBIN * L)

    k_a = np.asarray(k, np.float32).reshape(1, 1)
    th_a = np.asarray(thetao, np.float32).reshape(1, 1)
    in_maps = []
    for c in range(N_CORES):
        lo = c * R * 3
        sl = xyzf[lo: lo + XLEN]
        if sl.shape[0] < XLEN:
            sl = np.concatenate([sl, np.zeros(XLEN - sl.shape[0], np.float32)])
        in_maps.append({
            "xyz": np.ascontiguousarray(sl),
            "fl": np.ascontiguousarray(slots[c]),
            "k": k_a,
            "thetao": th_a,
        })
    return in_maps


def _reference_fallback(xyz, top, cell, k, thetao):
    xyz = np.asarray(xyz, np.float32)
    top = np.asarray(top)
    cell = np.asarray(cell, np.float32)

    def wrap(v):
        off = -(v >= 0.5 * cell).astype(np.float32) + (v < -0.5 * cell).astype(np.float32)
        return v + off * cell

    bv1 = wrap(xyz[top[:, 0]] - xyz[top[:, 1]])
    bv2 = wrap(xyz[top[:, 2]] - xyz[top[:, 1]])
    dot = np.sum(bv1 * bv2, axis=-1)
    norm = np.sqrt(np.sum(bv1 * bv1, axis=-1) * np.sum(bv2 * bv2, axis=-1))
    ang = np.arccos(dot / norm)
    return np.float32(0.5 * np.float32(k) * np.sum((ang - np.float32(thetao)) ** 2))


def kernel(xyz, top, cell, k, thetao):
    from concourse.bass_utils import run_bass_kernel_spmd

    xyz = np.asarray(xyz)
    top = np.asarray(top)
    cell = np.asarray(cell)
    structured = (
        xyz.shape == (N_ATOMS, 3)
        and top.shape == (N_ANGLES, 3)
        and np.allclose(np.asarray(cell, np.float64), 2 * BOXH)
        and bool(np.all(top[:, 1] == top[:, 0] + 1))
        and bool(np.all(top[:, 2] == top[:, 0] + 2))
    )
    if not structured:
        return _reference_fallback(xyz, top, cell, k, thetao)

    base = top[:, 0].astype(np.int64)
    in_maps = _shard_inputs(xyz, base, k, thetao)
    if in_maps is None:
        return _reference_fallback(xyz, top, cell, k, thetao)
    if _nc_cache[0] is None:
        _nc_cache[0] = _build()
    nc = _nc_cache[0]
    res = run_bass_kernel_spmd(nc, in_maps, core_ids=list(range(N_CORES)))
    total = np.float32(0.0)
    for c in range(N_CORES):
        total += np.float32(res.results[c]["out"][0, 0])
    return np.float32(total)


# revision 34
# speedup vs baseline: 1.1187x; 1.1187x over previous
"""AnglePotentials on 8 Trainium2 NeuronCores.

Math: for each angle (i, i+1, i+2) the energy term depends only on the base
atom index i, so we precompute on-device a per-atom table
    u[j] = (arccos(cos_angle(j)) - thetao)^2
from xyz (streamed, elementwise). Since bond vectors are shared between
neighbouring triplets (D[j] = xyz[j]-xyz[j+1] feeds both bv1[j] and bv2[j-1]),
one difference/wrap/square stream serves both bond vectors.

The energy is the sum of u over the 4M base indices. Rather than a
per-element gather (slow on this hardware: GPSIMD ap_gather ~27ns/idx,
indirect DMA descriptors are per-dest-row), angles are bucketed on the host
into W-atom cells (cell-list sharding); the device evaluates, for each
window position w in [0,W), one fused scalar_tensor_tensor op
    (slot == w) * u[cell, w]   with fused accumulate-sum (accum_out)
over all angle slots.

Sharding: atoms are range-partitioned over the 8 cores (262144 per core,
2048 per SBUF partition); each core computes a partial energy and the host
sums the 8 partials. Work that is off the DVE critical path (the bond
difference D and the wrap-sign combine) runs on GPSIMD; transcendentals and
squares run on the Scalar engine.
"""

import numpy as np

# ---------------------------------------------------------------- geometry
N_ATOMS = 2_000_000
N_ANGLES = 4_000_000
BOXH = 25.0              # half box
A = 2048                 # atoms per partition
R = 128 * A              # atoms per core = 262144
N_CORES = 8
W = 4                    # cell width (atoms)
NBIN = A // W            # cells per partition = 512
L = 25                   # angle slots per cell (fixed-seed max is 25)
SENT = 100.0             # sentinel slot value (never matches a window pos)
NSTRIPE = 4              # phase-1 stripes
SA = A // NSTRIPE        # atoms per partition per stripe = 1024
XLEN = 3 * (R + 2)       # xyz floats per core shard

_nc_cache = [None]
DEBUG_DUMP = False


def _build():
    import concourse.bass as bass
    import concourse.bacc as bacc
    import concourse.mybir as mybir
    import concourse.tile as tile

    AF = mybir.ActivationFunctionType
    ALU = mybir.AluOpType
    f32 = mybir.dt.float32
    bf16 = mybir.dt.bfloat16

    nc = bacc.Bacc("TRN2", target_bir_lowering=False, debug=False,
                   num_devices=N_CORES)
    xyz_d = nc.dram_tensor("xyz", [XLEN], f32, kind="ExternalInput").ap()
    fl_d = nc.dram_tensor("fl", [128, NBIN * L], bf16, kind="ExternalInput").ap()
    k_d = nc.dram_tensor("k", [1, 1], f32, kind="ExternalInput").ap()
    th_d = nc.dram_tensor("thetao", [1, 1], f32, kind="ExternalInput").ap()
    out_d = nc.dram_tensor("out", [1, 1], f32, kind="ExternalOutput").ap()
    if DEBUG_DUMP:
        dbg_u = nc.dram_tensor("dbg_u", [128, A], f32, kind="ExternalOutput").ap()

    with tile.TileContext(nc) as tc:
        with (
            tc.tile_pool(name="persist", bufs=1) as persist,
            tc.tile_pool(name="work", bufs=4) as work,
            tc.tile_pool(name="big", bufs=1) as big,
            tc.tile_pool(name="small", bufs=1) as small,
            tc.tile_pool(name="psum", bufs=1, space="PSUM") as psum,
        ):
            u = persist.tile([128, A], f32)
            fl = persist.tile([128, NBIN * L], bf16)

            # thetao -> per-partition bias = pi/2 - thetao
            th_s = persist.tile([1, 1], f32)
            nc.sync.dma_start(th_s[:], th_d[:])
            ones_row = persist.tile([1, 128], f32)
            nc.vector.memset(ones_row[:], 1.0)
            bias_ps = psum.tile([128, 1], f32)
            nc.tensor.matmul(bias_ps[:], ones_row[:], th_s[:], start=True, stop=True)
            bias_t = persist.tile([128, 1], f32)
            nc.vector.tensor_scalar(out=bias_t[:], in0=bias_ps[:],
                                    scalar1=-1.0, scalar2=float(np.pi / 2),
                                    op0=ALU.mult, op1=ALU.add)
            bias_m25 = persist.tile([128, 1], f32)
            nc.vector.memset(bias_m25[:], -BOXH)
            bias_p25 = persist.tile([128, 1], f32)
            nc.vector.memset(bias_p25[:], BOXH)

            cols = persist.tile([128, W], f32)

            # ---------------- phase 1: u[j] per stripe ----------------
            # D[j] = xyz[j] - xyz[j+1]; bv1[j] = wrap(D[j]); bv2[j] = -wrap(D[j+1])
            # dot' = sum_c Dw[j]*Dw[j+1] = -dot; nn[j] = |Dw[j]|^2
            # arccos = pi/2 + 2*arctan(dot'/(sqrt(q)+sqrt(m))), q = m - dot'^2
            for s in range(NSTRIPE):
                ND = 3 * SA + 3  # D elements (SA+1 atoms worth)
                X = work.tile([128, 3 * (SA + 2)], f32, tag="X")
                x_src = bass.AP(
                    tensor=xyz_d.tensor,
                    offset=s * 3 * SA,
                    ap=[[3 * A, 128], [1, 3 * (SA + 2)]],
                )
                nc.sync.dma_start(X[:], x_src)

                D = big.tile([128, ND], f32, tag="D")
                nc.gpsimd.tensor_tensor(out=D[:], in0=X[:, 0:ND],
                                        in1=X[:, 3:ND + 3], op=ALU.subtract)
                # wrap via sign: D - 25*(sign(D-25) + sign(D+25))
                s1 = big.tile([128, ND], f32, tag="s1")
                nc.scalar.activation(s1[:], D[:], AF.Sign, bias=bias_m25[:])
                s2 = big.tile([128, ND], f32, tag="s2")
                nc.scalar.activation(s2[:], D[:], AF.Sign, bias=bias_p25[:])
                nc.vector.tensor_tensor(out=s1[:], in0=s1[:], in1=s2[:], op=ALU.add)
                Db = big.tile([128, ND], f32, tag="Db")
                nc.vector.scalar_tensor_tensor(out=Db[:], in0=s1[:], scalar=-BOXH,
                                               in1=D[:], op0=ALU.mult, op1=ALU.add)
                P = big.tile([128, 3 * SA], f32, tag="P")
                nc.vector.tensor_tensor(out=P[:], in0=Db[:, 0:3 * SA],
                                        in1=Db[:, 3:3 * SA + 3], op=ALU.mult)
                S = big.tile([128, ND], f32, tag="S")
                nc.scalar.activation(S[:], Db[:], AF.Square)

                def comp_sum(src_t, n, tag):
                    o = small.tile([128, n], f32, tag=tag)
                    v = src_t[:, 0:3 * n].rearrange("p (a c) -> p a c", c=3)
                    nc.vector.tensor_reduce(out=o[:], in_=v[:, :, :],
                                            axis=mybir.AxisListType.X, op=ALU.add)
                    return o

                dot = comp_sum(P, SA, "dot")      # dot' = -dot
                nn = comp_sum(S, SA + 1, "nn")    # |Dw|^2 per atom

                m = small.tile([128, SA], f32, tag="m")
                nc.vector.tensor_tensor(out=m[:], in0=nn[:, 0:SA], in1=nn[:, 1:SA + 1],
                                        op=ALU.mult)
                d2 = small.tile([128, SA], f32, tag="d2")
                nc.vector.tensor_tensor(out=d2[:], in0=dot[:], in1=dot[:], op=ALU.mult)
                q = small.tile([128, SA], f32, tag="q")
                nc.vector.tensor_tensor(out=q[:], in0=m[:], in1=d2[:], op=ALU.subtract)
                nc.vector.tensor_scalar(out=q[:], in0=q[:], scalar1=0.0,
                                        scalar2=None, op0=ALU.max)
                sq = small.tile([128, SA], f32, tag="sq")
                nc.scalar.activation(sq[:], q[:], AF.Sqrt)
                sm = small.tile([128, SA], f32, tag="sm")
                nc.scalar.activation(sm[:], m[:], AF.Sqrt)
                den = small.tile([128, SA], f32, tag="den")
                nc.vector.tensor_tensor(out=den[:], in0=sq[:], in1=sm[:], op=ALU.add)
                nc.vector.tensor_scalar(out=den[:], in0=den[:], scalar1=1e-30,
                                        scalar2=None, op0=ALU.max)
                rec = small.tile([128, SA], f32, tag="rec")
                nc.vector.reciprocal_approx_fast(rec[:], den[:])
                rho = small.tile([128, SA], f32, tag="rho")
                nc.vector.tensor_tensor(out=rho[:], in0=dot[:], in1=rec[:], op=ALU.mult)
                at = small.tile([128, SA], f32, tag="at")
                nc.scalar.activation(at[:], rho[:], AF.Arctan)
                # u = (arccos - thetao)^2 = (2*at' + (pi/2 - thetao))^2
                nc.scalar.activation(u[:, s * SA:(s + 1) * SA], at[:], AF.Square,
                                     bias=bias_t[:], scale=2.0)

            # ---------------- phase 2: fused compare-select-accumulate ----
            dummy = persist.tile([128, NBIN * L], bf16)
            nc.sync.dma_start(fl[:], fl_d[:])
            fl2 = fl[:].rearrange("p (b t) -> p b t", t=L)
            u2 = u[:].rearrange("p (b w) -> p b w", w=W)
            d2v = dummy[:].rearrange("p (b t) -> p b t", t=L)
            for w in range(W):
                nc.vector.scalar_tensor_tensor(
                    out=d2v[:, :, :],
                    in0=fl2[:, :, :],
                    scalar=float(w),
                    in1=u2[:, :, w:w + 1].to_broadcast([128, NBIN, L]),
                    op0=ALU.is_equal,
                    op1=ALU.mult,
                    accum_out=cols[:, w:w + 1],
                )

            if DEBUG_DUMP:
                uf = persist.tile([128, A], f32)
                nc.vector.tensor_copy(uf[:], u[:])
                nc.sync.dma_start(dbg_u[:], uf[:])

            # ---------------- final reduction ----------------
            red = persist.tile([128, 1], f32)
            nc.vector.tensor_reduce(out=red[:], in_=cols[:],
                                    axis=mybir.AxisListType.X, op=ALU.add)
            ones_col = persist.tile([128, 1], f32)
            nc.vector.memset(ones_col[:], 1.0)
            tot_ps = psum.tile([1, 1], f32)
            nc.tensor.matmul(tot_ps[:], red[:], ones_col[:], start=True, stop=True)
            k_s = persist.tile([1, 1], f32)
            nc.sync.dma_start(k_s[:], k_d[:])
            tot = persist.tile([1, 1], f32)
            nc.vector.tensor_tensor(out=tot[:], in0=tot_ps[:], in1=k_s[:],
                                    op=ALU.mult)
            nc.vector.tensor_scalar(out=tot[:], in0=tot[:], scalar1=0.5,
                                    scalar2=None, op0=ALU.mult)
            nc.sync.dma_start(out_d[:], tot[:])
    nc.compile()
    return nc


def _shard_inputs(xyz, base, k, thetao):
    xyzf = np.ascontiguousarray(xyz, dtype=np.float32).reshape(-1)
    sb = np.sort(base.astype(np.int64), kind="stable")
    n = len(sb)
    nbins_g = N_CORES * 128 * NBIN
    edges = np.searchsorted(sb, np.arange(nbins_g + 1, dtype=np.int64) * W)
    g = sb // W
    r = np.arange(n, dtype=np.int64) - edges[g]
    if r.max(initial=0) >= L:
        return None  # cell overflow -> caller falls back
    import ml_dtypes
    bf = ml_dtypes.bfloat16
    slots = np.full((nbins_g, L), SENT, bf)
    slots[g, r] = (sb % W).astype(bf)
    slots = slots.reshape(N_CORES, 128, NBIN * L)

    k_a = np.asarray(k, np.float32).reshape(1, 1)
    th_a = np.asarray(thetao, np.float32).reshape(1, 1)
    in_maps = []
    for c in range(N_CORES):
        lo = c * R * 3
        sl = xyzf[lo: lo + XLEN]
        if sl.shape[0] < XLEN:
            sl = np.concatenate([sl, np.zeros(XLEN - sl.shape[0], np.float32)])
        in_maps.append({
            "xyz": np.ascontiguousarray(sl),
            "fl": np.ascontiguousarray(slots[c]),
            "k": k_a,
            "thetao": th_a,
        })
    return in_maps


def _reference_fallback(xyz, top, cell, k, thetao):
    xyz = np.asarray(xyz, np.float32)
    top = np.asarray(top)
    cell = np.asarray(cell, np.float32)

    def wrap(v):
        off = -(v >= 0.5 * cell).astype(np.float32) + (v < -0.5 * cell).astype(np.float32)
        return v + off * cell

    bv1 = wrap(xyz[top[:, 0]] - xyz[top[:, 1]])
    bv2 = wrap(xyz[top[:, 2]] - xyz[top[:, 1]])
    dot = np.sum(bv1 * bv2, axis=-1)
    norm = np.sqrt(np.sum(bv1 * bv1, axis=-1) * np.sum(bv2 * bv2, axis=-1))
    ang = np.arccos(dot / norm)
    return np.float32(0.5 * np.float32(k) * np.sum((ang - np.float32(thetao)) ** 2))


def kernel(xyz, top, cell, k, thetao):
    from concourse.bass_utils import run_bass_kernel_spmd

    xyz = np.asarray(xyz)
    top = np.asarray(top)
    cell = np.asarray(cell)
    structured = (
        xyz.shape == (N_ATOMS, 3)
        and top.shape == (N_ANGLES, 3)
        and np.allclose(np.asarray(cell, np.float64), 2 * BOXH)
        and bool(np.all(top[:, 1] == top[:, 0] + 1))
        and bool(np.all(top[:, 2] == top[:, 0] + 2))
    )
    if not structured:
        return _reference_fallback(xyz, top, cell, k, thetao)

    base = top[:, 0].astype(np.int64)
    in_maps = _shard_inputs(xyz, base, k, thetao)
    if in_maps is None:
        return _reference_fallback(xyz, top, cell, k, thetao)
    if _nc_cache[0] is None:
        _nc_cache[0] = _build()
    nc = _nc_cache[0]
    res = run_bass_kernel_spmd(nc, in_maps, core_ids=list(range(N_CORES)))
    total = np.float32(0.0)
    for c in range(N_CORES):
        total += np.float32(res.results[c]["out"][0, 0])
    return np.float32(total)
